# revision 16
# baseline (speedup 1.0000x reference)
"""Multi-head attention (softmax over the QUERY axis) on 8 TRN2 NeuronCores.

Problem shapes: Q [T=1024, B=8, D=256]; per-head projections Wq/Wk/Wv
[H=8, E=512, D=256]; Wo [D=256, H*E=4096]. Data-parallel over batch B.

Two exact algebraic restructurings (both exploit E > D):

1. V/output side: since o_h = attn_h @ v_h and v_h = x@Wv_h^T + bv_h,
       out = sum_h attn_h @ (x @ M_h^T + c_h) + bo,
       M_h = Wo_h @ Wv_h  (D x D, host),  c_h = bv_h @ Wo_h^T.
   Removes the V projection, the E-wide attn@V matmul and the output
   projection.

2. Q/K side: q_t . k_s = x_t . (G_h x_s) with G_h = Wq_h^T @ Wk_h
   (D x D, host).  The softmax is over the QUERY axis t, so per-key
   additive terms (bq.k_s, bq.bk) cancel EXACTLY and only
   w1_h = s*Wq_h^T @ bk_h survives as a bias on the z projection:
       lg[t,s] = x_t . z_s,   z = x @ (s*G_h) + w1_h.
   Removes both the q and k projections; scores contract over D=256
   instead of E=512.

Per-head MACs: 1611M -> 670M.  The scores matmul runs in fp8 (e4m3)
DoubleRow (z8 x x8, both cast with power-of-2 scales); the attention
output Pn^T x Ex runs in bf16 (Ex = exp from the ScalarE with the
softmax denominators l[s] from its accum_out; Pn = (x@M^T + c)*AP/l).

The head loop is software-pipelined two-deep: head h's scores matmuls
are interleaved with head h-1's AV matmuls and head h+1's z/P
projection matmuls, so the PE never head-of-line blocks on the ACT exp
pace and never idles while the exp -> l -> rr -> Pn chain drains
(PE-idle gaps >3.4us re-throttle the HAM clock gate to half rate).
"""

import sys

sys.path.insert(0, "/opt/trn_rl_repo")

from contextlib import ExitStack

import ml_dtypes
import numpy as np

import concourse.bass as bass
import concourse.tile as tile
from concourse.tile import add_dep_helper
from concourse import bacc, bass_utils, mybir

T, B, D, H, E = 1024, 8, 256, 8, 512
N_CORES = 8
AX = 8.0        # fp8 scale on x8 (folded into qt host-side)
AZ = 128.0      # fp8 scale on z8 (logit psum = AX*AZ*lg)
AP = 8192.0     # scale on Pn / out_acc

F32 = mybir.dt.float32
BF16 = mybir.dt.bfloat16
F8 = mybir.dt.float8e4
AF = mybir.ActivationFunctionType
ALU = mybir.AluOpType
DR = mybir.MatmulPerfMode.DoubleRow


def _bcast(ap_row, parts):
    """Partition-broadcast a [1, n] DRAM AP to [parts, n] (step-0 partition)."""
    return bass.AP(
        tensor=ap_row.tensor,
        offset=ap_row.offset,
        ap=[[0, parts], list(ap_row.ap[-1])],
    )


def build_nc(t=T, d=D, h=H, e=E):
    """Build the per-core SPMD program. Returns a compiled Bacc."""
    TC = t // 512   # t chunks (512-wide psum free dim)
    SB = t // 128   # s blocks
    DC = d // 128   # d chunks (contraction for projections)
    DB = d // 128   # d blocks (z free dim / transposed-output partitions)

    sc = 1.0 / (AX * AZ)

    nc = bacc.Bacc("TRN2", target_bir_lowering=False, debug=False)

    qt_d = nc.dram_tensor("qt", [128, DC, t], BF16, kind="ExternalInput").ap()
    gz_d = nc.dram_tensor("gz", [h, 128, DC, d], BF16, kind="ExternalInput").ap()
    mt_d = nc.dram_tensor("mt", [h, 128, DC, d], BF16, kind="ExternalInput").ap()
    w1_d = nc.dram_tensor("w1s", [128, h, DB], F32, kind="ExternalInput").ap()
    cs_d = nc.dram_tensor("cs", [h, d], F32, kind="ExternalInput").ap()
    boc_d = nc.dram_tensor("boc", [128, DB], F32, kind="ExternalInput").ap()
    out_d = nc.dram_tensor("out", [d, t], F32, kind="ExternalOutput").ap()

    with tile.TileContext(nc) as tc, ExitStack() as ctx:
        consts = ctx.enter_context(tc.tile_pool(name="consts", bufs=1))
        wpool = ctx.enter_context(tc.tile_pool(name="wpool", bufs=2))
        hpool = ctx.enter_context(tc.tile_pool(name="hpool", bufs=2))
        qkpool = ctx.enter_context(tc.tile_pool(name="qkpool", bufs=2))
        spool = ctx.enter_context(tc.tile_pool(name="spool", bufs=2))
        at_pool = ctx.enter_context(tc.tile_pool(name="at_pool", bufs=2, space="PSUM"))
        mm_pool = ctx.enter_context(tc.tile_pool(name="mm_pool", bufs=6, space="PSUM"))

        # ---- persistent loads -------------------------------------------
        qt_sb = consts.tile([128, DC, t], BF16)
        nc.sync.dma_start(out=qt_sb[:, 0, :], in_=qt_d[:, 0, :])
        w1_sb = consts.tile([128, h, DB], F32)
        nc.sync.dma_start(out=w1_sb, in_=w1_d)
        boc_sb = consts.tile([128, DB], F32)
        nc.sync.dma_start(out=boc_sb, in_=boc_d)
        out_acc = consts.tile([128, DB, t], F32)
        out_r = out_d.rearrange("(db p) t -> p db t", p=128)

        # ---- PE warm-up: dummy matmuls during the initial DMA wait ------
        scratch = consts.tile([128, 640], BF16)
        nc.vector.memset(scratch, 0.0)
        ps_w = mm_pool.tile([128, 512], F32, tag="mm")
        for _ in range(6):
            nc.tensor.matmul(
                ps_w, scratch[:, :128], scratch[:, 128:640], start=True, stop=True
            )

        gz_sb = [None] * h
        mt_sb = [None] * h
        c_bc = [None] * h
        zT8 = [None] * h
        P32 = [None] * h
        Pnb = [None] * h
        Ex = [None] * h

        def load_head(hh, gate_mm=None):
            gz_sb[hh] = wpool.tile([128, DC, d], BF16, tag="gz", name="gz_sb")
            nc.sync.dma_start(out=gz_sb[hh], in_=gz_d[hh])
            mt_sb[hh] = wpool.tile([128, DC, d], BF16, tag="mt", name="mt_sb")
            mm = nc.sync.dma_start(out=mt_sb[hh], in_=mt_d[hh])
            c_bc[hh] = wpool.tile([128, d], F32, tag="c", name="c_bc")
            cc = nc.gpsimd.dma_start(
                out=c_bc[hh], in_=_bcast(cs_d[hh][None, :], 128)
            )
            if gate_mm is not None:
                for g in (mm, cc):
                    add_dep_helper(
                        g.ins, gate_mm.ins, reason="defer bulk load past cold start"
                    )

        def z_group(hh, db, tch):
            """One psum-group of the z projection: z = x@(s*G) + w1 -> fp8."""
            tsl = slice(tch * 512, (tch + 1) * 512)
            ps_z = mm_pool.tile([128, 512], F32, tag="mm")
            first = None
            for dc in range(DC):
                mm = nc.tensor.matmul(
                    ps_z,
                    gz_sb[hh][:, dc, db * 128 : (db + 1) * 128],
                    qt_sb[:, dc, tsl],
                    start=(dc == 0),
                    stop=(dc == DC - 1),
                )
                first = first or mm
            nc.vector.tensor_scalar_add(
                zT8[hh][:, db, tsl], ps_z, w1_sb[:, hh, db : db + 1]
            )
            return first

        def p_group(hh, sb):
            """One psum-group of the P projection: P32 = x @ M^T + c."""
            ssl = slice(sb * 128, (sb + 1) * 128)
            pp = mm_pool.tile([128, 512], F32, tag="mm")
            for dc in range(DC):
                nc.tensor.matmul(
                    pp[:, :d],
                    qt_sb[:, dc, ssl],
                    mt_sb[hh][:, dc, :],
                    start=(dc == 0),
                    stop=(dc == DC - 1),
                )
            nc.vector.tensor_add(P32[hh][:, sb, :], pp[:, :d], c_bc[hh])

        def av_group(hh, dt, tch):
            """One psum-group of the AV matmul (bf16): out^T += Pn^T x Ex."""
            dsl = slice(dt * 128, (dt + 1) * 128)
            tsl = slice(tch * 512, (tch + 1) * 512)
            ot = mm_pool.tile([128, 512], F32, tag="mm")
            for sb in range(SB):
                nc.tensor.matmul(
                    ot,
                    Pnb[hh][:, sb, dsl],
                    Ex[hh][:, sb, tsl],
                    start=(sb == 0),
                    stop=(sb == SB - 1),
                )
            if hh == 0:
                nc.scalar.activation(out_acc[:, dt, tsl], ot, AF.Copy)
            else:
                nc.vector.tensor_add(out_acc[:, dt, tsl], out_acc[:, dt, tsl], ot)

        # ---- prologue: head 0 z/P projections + the shared x8 cast ------
        load_head(0)
        nc.sync.dma_start(out=qt_sb[:, 1, :], in_=qt_d[:, 1, :])
        zT8[0] = qkpool.tile([128, DB, t], F8, tag="zT", name="zT8")
        first_mm0 = None
        for db in range(DB):
            for tch in range(TC):
                mm = z_group(0, db, tch)
                first_mm0 = first_mm0 or mm
        x8 = consts.tile([128, DC, t], F8)
        for dc in range(DC):
            nc.vector.tensor_scalar_mul(x8[:, dc, :], qt_sb[:, dc, :], 1.0)
        P32[0] = hpool.tile([128, SB, d], F32, tag="P32", name="P32")
        for sb in range(SB):
            p_group(0, sb)

        for hh in range(h):
            if hh + 1 < h:
                load_head(hh + 1, gate_mm=first_mm0 if hh == 0 else None)
                zT8[hh + 1] = qkpool.tile([128, DB, t], F8, tag="zT", name="zT8")
                P32[hh + 1] = hpool.tile([128, SB, d], F32, tag="P32", name="P32")

            # filler matmul groups to interleave with this head's scores:
            # previous head's AV + next head's z/P projections
            fillers = []
            if hh > 0:
                for dt in range(DB):
                    for tch in range(TC):
                        fillers.append(lambda dt=dt, tch=tch: av_group(hh - 1, dt, tch))
            if hh + 1 < h:
                for db in range(DB):
                    for tch in range(TC):
                        fillers.append(lambda db=db, tch=tch: z_group(hh + 1, db, tch))
                for sb in range(SB):
                    fillers.append(lambda sb=sb: p_group(hh + 1, sb))

            # ---- scores (fp8 DR) -> exp on ACT (+accum l), interleaved --
            Ex[hh] = hpool.tile([128, SB, t], BF16, tag="Ex", name="Ex")
            lsum2 = spool.tile([128, SB, TC], F32)
            fi = 0
            for sb in range(SB):
                ssl = slice(sb * 128, (sb + 1) * 128)
                for tch in range(TC):
                    tsl = slice(tch * 512, (tch + 1) * 512)
                    at = at_pool.tile([128, 512], F32, tag="at")
                    nc.tensor.matmul(
                        at,
                        zT8[hh][:, :, ssl],
                        x8[:, :, tsl],
                        start=True,
                        stop=True,
                        perf_mode=DR,
                    )
                    nc.scalar.activation(
                        Ex[hh][:, sb, tsl],
                        at,
                        AF.Exp,
                        scale=sc,
                        accum_out=lsum2[:, sb, tch : tch + 1],
                    )
                for _ in range(2):
                    if fi < len(fillers):
                        fillers[fi]()
                        fi += 1
            while fi < len(fillers):
                fillers[fi]()
                fi += 1

            # ---- softmax denominators: rr2 = AP / l ---------------------
            ls = spool.tile([128, SB], F32)
            lsS = spool.tile([128, SB], F32)
            rr2 = spool.tile([128, SB], F32)
            nc.vector.tensor_add(ls, lsum2[:, :, 0], lsum2[:, :, 1])
            nc.vector.tensor_scalar_mul(lsS, ls, 1.0 / AP)
            nc.vector.reciprocal(rr2, lsS)

            # ---- Pnb (bf16 stationary operand of AV) on DVE -------------
            Pnb[hh] = hpool.tile([128, SB, d], BF16, tag="Pnb", name="Pnb")
            for sb in range(SB):
                nc.vector.tensor_scalar_mul(
                    Pnb[hh][:, sb, :], P32[hh][:, sb, :], rr2[:, sb : sb + 1]
                )

        # ---- epilogue: last head's AV -----------------------------------
        for dt in range(DB):
            for tch in range(TC):
                av_group(h - 1, dt, tch)

        # ---- final: out = (out_acc + AP*bo) / AP, store -----------------
        for dt in range(DB):
            nc.vector.tensor_scalar(
                out_acc[:, dt, :],
                out_acc[:, dt, :],
                boc_sb[:, dt : dt + 1],
                1.0 / AP,
                op0=ALU.add,
                op1=ALU.mult,
            )
            nc.sync.dma_start(out=out_r[:, dt, :], in_=out_acc[:, dt, :])

    nc.compile()
    return nc


_NC_CACHE = {}


def _get_nc(shape_key):
    if shape_key not in _NC_CACHE:
        _NC_CACHE[shape_key] = build_nc(*shape_key)
    return _NC_CACHE[shape_key]


def _pmajor(a, last):
    """[..., C*128, last] -> [..., 128, C, last] partition-major layout."""
    lead = a.shape[:-2]
    c = a.shape[-2] // 128
    return np.ascontiguousarray(
        a.reshape(*lead, c, 128, last).swapaxes(-3, -2)
    )


def _prep_inputs(Q, Wq, bq, Wk, bk, Wv, bv, Wo, bo):
    t, b, d = Q.shape
    h, e, _ = Wq.shape
    s = np.float32(1.0 / np.sqrt(e))
    bf = ml_dtypes.bfloat16
    Q = np.asarray(Q, np.float32)
    Wq = np.asarray(Wq, np.float32)
    Wk = np.asarray(Wk, np.float32)
    Wv = np.asarray(Wv, np.float32)
    Wo = np.asarray(Wo, np.float32)
    bk = np.asarray(bk, np.float32)
    bv = np.asarray(bv, np.float32)
    bo = np.asarray(bo, np.float32)
    # [B, 128, DC, T] partition-major AX * x^T per batch
    qt_all = _pmajor((Q * AX).transpose(1, 2, 0).astype(bf), t)
    # z side: Gz = (AZ*s/AX) * (Wk^T @ Wq),  w1 = AZ*s*Wq^T@bk
    gzs = np.stack([(AZ * s / AX) * (Wk[hh].T @ Wq[hh]) for hh in range(h)])
    gz = _pmajor(gzs.astype(bf), d)
    w1 = np.stack([(AZ * s) * (Wq[hh].T @ bk[hh]) for hh in range(h)])
    # P side: M_h = Wo_h @ Wv_h; mt stores M_h^T/AX partition-major over d'
    Wo_heads = Wo.reshape(d, h, e)
    mts = np.stack([(Wo_heads[:, hh, :] @ Wv[hh]).T / AX for hh in range(h)])
    mt = _pmajor(mts.astype(bf), d)
    cs = np.stack([bv[hh] @ Wo_heads[:, hh, :].T for hh in range(h)])
    shared = {
        "gz": gz,
        "mt": mt,
        "w1s": np.ascontiguousarray(w1.reshape(h, -1, 128).transpose(2, 0, 1)),
        "cs": np.ascontiguousarray(cs.astype(np.float32)),
        "boc": np.ascontiguousarray((bo * AP).reshape(-1, 128).T.astype(np.float32)),
    }
    in_maps = [
        {"qt": np.ascontiguousarray(qt_all[bb]), **shared} for bb in range(b)
    ]
    return in_maps, (t, d, h, e)


def kernel(Q, Wq, bq, Wk, bk, Wv, bv, Wo, bo, _trace=False):
    in_maps, (t, d, h, e) = _prep_inputs(Q, Wq, bq, Wk, bk, Wv, bv, Wo, bo)
    nc = _get_nc((t, d, h, e))
    res = bass_utils.run_bass_kernel_spmd(
        nc, in_maps, core_ids=list(range(len(in_maps))), trace=_trace
    )
    # per-core output is out^T [D, T]; transpose back and stack over batch
    out = np.stack(
        [res.results[bb]["out"].T for bb in range(len(in_maps))], axis=1
    )
    if _trace:
        kernel.last_results = res
    return np.ascontiguousarray(out.astype(np.float32))


# revision 17
# speedup vs baseline: 1.1626x; 1.1626x over previous
"""Multi-head attention (softmax over the QUERY axis) on 8 TRN2 NeuronCores.

Problem shapes: Q [T=1024, B=8, D=256]; per-head projections Wq/Wk/Wv
[H=8, E=512, D=256]; Wo [D=256, H*E=4096]. Data-parallel over batch B.

Two exact algebraic restructurings (both exploit E > D):

1. V/output side: since o_h = attn_h @ v_h and v_h = x@Wv_h^T + bv_h,
       out = sum_h attn_h @ (x @ M_h^T + c_h) + bo,
       M_h = Wo_h @ Wv_h  (D x D, host),  c_h = bv_h @ Wo_h^T.
   Removes the V projection, the E-wide attn@V matmul and the output
   projection.

2. Q/K side: q_t . k_s = x_t . (G_h x_s) with G_h = Wq_h^T @ Wk_h
   (D x D, host).  The softmax is over the QUERY axis t, so per-key
   additive terms (bq.k_s, bq.bk) cancel EXACTLY and only
   w1_h = s*Wq_h^T @ bk_h survives as a bias on the z projection:
       lg[t,s] = x_t . z_s,   z = x @ (s*G_h) + w1_h.
   Removes both the q and k projections; scores contract over D=256
   instead of E=512.

Per-head MACs: 1611M -> 670M.  The scores matmul runs in fp8 (e4m3)
DoubleRow (z8 x x8, both cast with power-of-2 scales); the attention
output Pn^T x Ex runs in bf16 (Ex = exp from the ScalarE with the
softmax denominators l[s] from its accum_out; Pn = (x@M^T + c)*AP/l).

The head loop is software-pipelined two-deep: head h's scores matmuls
are interleaved with head h-1's AV matmuls and head h+1's z/P
projection matmuls, so the PE never head-of-line blocks on the ACT exp
pace and never idles while the exp -> l -> rr -> Pn chain drains
(PE-idle gaps >3.4us re-throttle the HAM clock gate to half rate).
"""

import sys

sys.path.insert(0, "/opt/trn_rl_repo")

from contextlib import ExitStack

import ml_dtypes
import numpy as np

import concourse.bass as bass
import concourse.tile as tile
from concourse.tile import add_dep_helper
from concourse import bacc, bass_utils, mybir

T, B, D, H, E = 1024, 8, 256, 8, 512
N_CORES = 8
AX = 8.0        # fp8 scale on x8 (folded into qt host-side)
AZ = 128.0      # fp8 scale on z8 (logit psum = AX*AZ*lg)
AP = 8192.0     # scale on Pn / out_acc

F32 = mybir.dt.float32
BF16 = mybir.dt.bfloat16
F8 = mybir.dt.float8e4
AF = mybir.ActivationFunctionType
ALU = mybir.AluOpType
DR = mybir.MatmulPerfMode.DoubleRow


def _bcast(ap_row, parts):
    """Partition-broadcast a [1, n] DRAM AP to [parts, n] (step-0 partition)."""
    return bass.AP(
        tensor=ap_row.tensor,
        offset=ap_row.offset,
        ap=[[0, parts], list(ap_row.ap[-1])],
    )


def build_nc(t=T, d=D, h=H, e=E):
    """Build the per-core SPMD program. Returns a compiled Bacc."""
    TC = t // 512   # t chunks (512-wide psum free dim)
    SB = t // 128   # s blocks
    DC = d // 128   # d chunks (contraction for projections)
    DB = d // 128   # d blocks (z free dim / transposed-output partitions)

    sc = 1.0 / (AX * AZ)

    nc = bacc.Bacc("TRN2", target_bir_lowering=False, debug=False)

    qt_d = nc.dram_tensor("qt", [128, DC, t], BF16, kind="ExternalInput").ap()
    gz_d = nc.dram_tensor("gz", [h, 128, DC, d], BF16, kind="ExternalInput").ap()
    mt_d = nc.dram_tensor("mt", [h, 128, DC, d], BF16, kind="ExternalInput").ap()
    w1_d = nc.dram_tensor("w1s", [128, h, DB], F32, kind="ExternalInput").ap()
    cs_d = nc.dram_tensor("cs", [h, d], F32, kind="ExternalInput").ap()
    boc_d = nc.dram_tensor("boc", [128, DB], F32, kind="ExternalInput").ap()
    out_d = nc.dram_tensor("out", [d, t], F32, kind="ExternalOutput").ap()

    with tile.TileContext(nc) as tc, ExitStack() as ctx:
        consts = ctx.enter_context(tc.tile_pool(name="consts", bufs=1))
        wpool = ctx.enter_context(tc.tile_pool(name="wpool", bufs=2))
        hpool = ctx.enter_context(tc.tile_pool(name="hpool", bufs=2))
        qkpool = ctx.enter_context(tc.tile_pool(name="qkpool", bufs=2))
        spool = ctx.enter_context(tc.tile_pool(name="spool", bufs=2))
        at_pool = ctx.enter_context(tc.tile_pool(name="at_pool", bufs=3, space="PSUM"))
        mm_pool = ctx.enter_context(tc.tile_pool(name="mm_pool", bufs=5, space="PSUM"))

        # ---- persistent loads -------------------------------------------
        qt_sb = consts.tile([128, DC, t], BF16)
        nc.sync.dma_start(out=qt_sb[:, 0, :], in_=qt_d[:, 0, :])
        w1_sb = consts.tile([128, h, DB], F32)
        nc.sync.dma_start(out=w1_sb, in_=w1_d)
        boc_sb = consts.tile([128, DB], F32)
        nc.sync.dma_start(out=boc_sb, in_=boc_d)
        out_acc = consts.tile([128, DB, t], F32)
        out_r = out_d.rearrange("(db p) t -> p db t", p=128)

        # ---- PE warm-up: dummy matmuls during the initial DMA wait ------
        scratch = consts.tile([128, 640], BF16)
        nc.vector.memset(scratch, 0.0)
        ps_w = mm_pool.tile([128, 512], F32, tag="mm")
        for _ in range(6):
            nc.tensor.matmul(
                ps_w, scratch[:, :128], scratch[:, 128:640], start=True, stop=True
            )

        gz_sb = [None] * h
        mt_sb = [None] * h
        c_bc = [None] * h
        zT8 = [None] * h
        P32 = [None] * h
        Pnb = [None] * h
        Ex = [None] * h

        def load_head(hh, gate_mm=None):
            gz_sb[hh] = wpool.tile([128, DC, d], BF16, tag="gz", name="gz_sb")
            nc.sync.dma_start(out=gz_sb[hh], in_=gz_d[hh])
            mt_sb[hh] = wpool.tile([128, DC, d], BF16, tag="mt", name="mt_sb")
            mm = nc.sync.dma_start(out=mt_sb[hh], in_=mt_d[hh])
            c_bc[hh] = wpool.tile([128, d], F32, tag="c", name="c_bc")
            cc = nc.gpsimd.dma_start(
                out=c_bc[hh], in_=_bcast(cs_d[hh][None, :], 128)
            )
            if gate_mm is not None:
                for g in (mm, cc):
                    add_dep_helper(
                        g.ins, gate_mm.ins, reason="defer bulk load past cold start"
                    )

        def z_group(hh, db, tch):
            """One psum-group of the z projection: z = x@(s*G) + w1 -> fp8."""
            tsl = slice(tch * 512, (tch + 1) * 512)
            ps_z = mm_pool.tile([128, 512], F32, tag="mm")
            first = None
            for dc in range(DC):
                mm = nc.tensor.matmul(
                    ps_z,
                    gz_sb[hh][:, dc, db * 128 : (db + 1) * 128],
                    qt_sb[:, dc, tsl],
                    start=(dc == 0),
                    stop=(dc == DC - 1),
                )
                first = first or mm
            nc.vector.tensor_scalar_add(
                zT8[hh][:, db, tsl], ps_z, w1_sb[:, hh, db : db + 1]
            )
            return first

        def p_group(hh, sb):
            """One psum-group of the P projection: P32 = x @ M^T + c."""
            ssl = slice(sb * 128, (sb + 1) * 128)
            pp = mm_pool.tile([128, 512], F32, tag="mm")
            for dc in range(DC):
                nc.tensor.matmul(
                    pp[:, :d],
                    qt_sb[:, dc, ssl],
                    mt_sb[hh][:, dc, :],
                    start=(dc == 0),
                    stop=(dc == DC - 1),
                )
            nc.vector.tensor_add(P32[hh][:, sb, :], pp[:, :d], c_bc[hh])

        def av_group(hh, dt, tch):
            """One psum-group of the AV matmul (bf16): out^T += Pn^T x Ex."""
            dsl = slice(dt * 128, (dt + 1) * 128)
            tsl = slice(tch * 512, (tch + 1) * 512)
            ot = mm_pool.tile([128, 512], F32, tag="mm")
            for sb in range(SB):
                nc.tensor.matmul(
                    ot,
                    Pnb[hh][:, sb, dsl],
                    Ex[hh][:, sb, tsl],
                    start=(sb == 0),
                    stop=(sb == SB - 1),
                )
            if hh == 0:
                nc.scalar.activation(out_acc[:, dt, tsl], ot, AF.Copy)
            else:
                nc.vector.tensor_add(out_acc[:, dt, tsl], out_acc[:, dt, tsl], ot)

        # ---- prologue: head 0 z/P projections + the shared x8 cast ------
        load_head(0)
        nc.sync.dma_start(out=qt_sb[:, 1, :], in_=qt_d[:, 1, :])
        zT8[0] = qkpool.tile([128, DB, t], F8, tag="zT", name="zT8")
        first_mm0 = None
        for db in range(DB):
            for tch in range(TC):
                mm = z_group(0, db, tch)
                first_mm0 = first_mm0 or mm
        x8 = consts.tile([128, DC, t], F8)
        for dc in range(DC):
            nc.vector.tensor_scalar_mul(x8[:, dc, :], qt_sb[:, dc, :], 1.0)
        P32[0] = hpool.tile([128, SB, d], F32, tag="P32", name="P32")
        for sb in range(SB):
            p_group(0, sb)

        for hh in range(h):
            if hh + 1 < h:
                load_head(hh + 1, gate_mm=first_mm0 if hh == 0 else None)
                zT8[hh + 1] = qkpool.tile([128, DB, t], F8, tag="zT", name="zT8")
                P32[hh + 1] = hpool.tile([128, SB, d], F32, tag="P32", name="P32")

            # filler matmul groups to interleave with this head's scores:
            # previous head's AV + next head's z/P projections
            big = []     # AV groups: 8 matmuls each
            small = []   # z/P projection groups: 2 matmuls each
            if hh > 0:
                for dt in range(DB):
                    for tch in range(TC):
                        big.append(lambda dt=dt, tch=tch: av_group(hh - 1, dt, tch))
            if hh + 1 < h:
                for db in range(DB):
                    for tch in range(TC):
                        small.append(lambda db=db, tch=tch: z_group(hh + 1, db, tch))
                for sb in range(SB):
                    small.append(lambda sb=sb: p_group(hh + 1, sb))
            # weave big groups evenly among the small ones so every score
            # round gets enough PE filler to cover the ACT exp pace
            fillers = []
            nb, ns = len(big), len(small)
            bi = si = 0
            for k in range(nb + ns):
                if bi * (nb + ns) <= k * nb and bi < nb:
                    fillers.append(big[bi]); bi += 1
                elif si < ns:
                    fillers.append(small[si]); si += 1
                else:
                    fillers.append(big[bi]); bi += 1

            # ---- scores (fp8 DR) -> exp on ACT (+accum l), interleaved --
            Ex[hh] = hpool.tile([128, SB, t], BF16, tag="Ex", name="Ex")
            lsum2 = spool.tile([128, SB, TC], F32)
            fi = 0
            for sb in range(SB):
                ssl = slice(sb * 128, (sb + 1) * 128)
                for tch in range(TC):
                    tsl = slice(tch * 512, (tch + 1) * 512)
                    at = at_pool.tile([128, 512], F32, tag="at")
                    nc.tensor.matmul(
                        at,
                        zT8[hh][:, :, ssl],
                        x8[:, :, tsl],
                        start=True,
                        stop=True,
                        perf_mode=DR,
                    )
                    nc.scalar.activation(
                        Ex[hh][:, sb, tsl],
                        at,
                        AF.Exp,
                        scale=sc,
                        accum_out=lsum2[:, sb, tch : tch + 1],
                    )
                for _ in range(2):
                    if fi < len(fillers):
                        fillers[fi]()
                        fi += 1
            while fi < len(fillers):
                fillers[fi]()
                fi += 1

            # ---- softmax denominators: rr2 = AP / l ---------------------
            ls = spool.tile([128, SB], F32)
            lsS = spool.tile([128, SB], F32)
            rr2 = spool.tile([128, SB], F32)
            nc.vector.tensor_add(ls, lsum2[:, :, 0], lsum2[:, :, 1])
            nc.vector.tensor_scalar_mul(lsS, ls, 1.0 / AP)
            nc.vector.reciprocal(rr2, lsS)

            # ---- Pnb (bf16 stationary operand of AV) on DVE -------------
            Pnb[hh] = hpool.tile([128, SB, d], BF16, tag="Pnb", name="Pnb")
            for sb in range(SB):
                nc.vector.tensor_scalar_mul(
                    Pnb[hh][:, sb, :], P32[hh][:, sb, :], rr2[:, sb : sb + 1]
                )

        # ---- epilogue: last head's AV -----------------------------------
        for dt in range(DB):
            for tch in range(TC):
                av_group(h - 1, dt, tch)

        # ---- final: out = (out_acc + AP*bo) / AP, store -----------------
        for dt in range(DB):
            nc.vector.tensor_scalar(
                out_acc[:, dt, :],
                out_acc[:, dt, :],
                boc_sb[:, dt : dt + 1],
                1.0 / AP,
                op0=ALU.add,
                op1=ALU.mult,
            )
            nc.sync.dma_start(out=out_r[:, dt, :], in_=out_acc[:, dt, :])

    nc.compile()
    return nc


_NC_CACHE = {}


def _get_nc(shape_key):
    if shape_key not in _NC_CACHE:
        _NC_CACHE[shape_key] = build_nc(*shape_key)
    return _NC_CACHE[shape_key]


def _pmajor(a, last):
    """[..., C*128, last] -> [..., 128, C, last] partition-major layout."""
    lead = a.shape[:-2]
    c = a.shape[-2] // 128
    return np.ascontiguousarray(
        a.reshape(*lead, c, 128, last).swapaxes(-3, -2)
    )


def _prep_inputs(Q, Wq, bq, Wk, bk, Wv, bv, Wo, bo):
    t, b, d = Q.shape
    h, e, _ = Wq.shape
    s = np.float32(1.0 / np.sqrt(e))
    bf = ml_dtypes.bfloat16
    Q = np.asarray(Q, np.float32)
    Wq = np.asarray(Wq, np.float32)
    Wk = np.asarray(Wk, np.float32)
    Wv = np.asarray(Wv, np.float32)
    Wo = np.asarray(Wo, np.float32)
    bk = np.asarray(bk, np.float32)
    bv = np.asarray(bv, np.float32)
    bo = np.asarray(bo, np.float32)
    # [B, 128, DC, T] partition-major AX * x^T per batch
    qt_all = _pmajor((Q * AX).transpose(1, 2, 0).astype(bf), t)
    # z side: Gz = (AZ*s/AX) * (Wk^T @ Wq),  w1 = AZ*s*Wq^T@bk
    gzs = np.stack([(AZ * s / AX) * (Wk[hh].T @ Wq[hh]) for hh in range(h)])
    gz = _pmajor(gzs.astype(bf), d)
    w1 = np.stack([(AZ * s) * (Wq[hh].T @ bk[hh]) for hh in range(h)])
    # P side: M_h = Wo_h @ Wv_h; mt stores M_h^T/AX partition-major over d'
    Wo_heads = Wo.reshape(d, h, e)
    mts = np.stack([(Wo_heads[:, hh, :] @ Wv[hh]).T / AX for hh in range(h)])
    mt = _pmajor(mts.astype(bf), d)
    cs = np.stack([bv[hh] @ Wo_heads[:, hh, :].T for hh in range(h)])
    shared = {
        "gz": gz,
        "mt": mt,
        "w1s": np.ascontiguousarray(w1.reshape(h, -1, 128).transpose(2, 0, 1)),
        "cs": np.ascontiguousarray(cs.astype(np.float32)),
        "boc": np.ascontiguousarray((bo * AP).reshape(-1, 128).T.astype(np.float32)),
    }
    in_maps = [
        {"qt": np.ascontiguousarray(qt_all[bb]), **shared} for bb in range(b)
    ]
    return in_maps, (t, d, h, e)


def kernel(Q, Wq, bq, Wk, bk, Wv, bv, Wo, bo, _trace=False):
    in_maps, (t, d, h, e) = _prep_inputs(Q, Wq, bq, Wk, bk, Wv, bv, Wo, bo)
    nc = _get_nc((t, d, h, e))
    res = bass_utils.run_bass_kernel_spmd(
        nc, in_maps, core_ids=list(range(len(in_maps))), trace=_trace
    )
    # per-core output is out^T [D, T]; transpose back and stack over batch
    out = np.stack(
        [res.results[bb]["out"].T for bb in range(len(in_maps))], axis=1
    )
    if _trace:
        kernel.last_results = res
    return np.ascontiguousarray(out.astype(np.float32))


# revision 18
# speedup vs baseline: 1.1917x; 1.0250x over previous
"""Multi-head attention (softmax over the QUERY axis) on 8 TRN2 NeuronCores.

Problem shapes: Q [T=1024, B=8, D=256]; per-head projections Wq/Wk/Wv
[H=8, E=512, D=256]; Wo [D=256, H*E=4096]. Data-parallel over batch B.

Two exact algebraic restructurings (both exploit E > D):

1. V/output side: since o_h = attn_h @ v_h and v_h = x@Wv_h^T + bv_h,
       out = sum_h attn_h @ (x @ M_h^T + c_h) + bo,
       M_h = Wo_h @ Wv_h  (D x D, host),  c_h = bv_h @ Wo_h^T.
   Removes the V projection, the E-wide attn@V matmul and the output
   projection.

2. Q/K side: q_t . k_s = x_t . (G_h x_s) with G_h = Wq_h^T @ Wk_h
   (D x D, host).  The softmax is over the QUERY axis t, so per-key
   additive terms (bq.k_s, bq.bk) cancel EXACTLY and only
   w1_h = s*Wq_h^T @ bk_h survives as a bias on the z projection:
       lg[t,s] = x_t . z_s,   z = x @ (s*G_h) + w1_h.
   Removes both the q and k projections; scores contract over D=256
   instead of E=512.

Per-head MACs: 1611M -> 670M.  The scores matmul runs in fp8 (e4m3)
DoubleRow (z8 x x8, both cast with power-of-2 scales); the attention
output Pn^T x Ex runs in bf16 (Ex = exp from the ScalarE with the
softmax denominators l[s] from its accum_out; Pn = (x@M^T + c)*AP/l).

The head loop is software-pipelined two-deep: head h's scores matmuls
are interleaved with head h-1's AV matmuls and head h+1's z/P
projection matmuls, so the PE never head-of-line blocks on the ACT exp
pace and never idles while the exp -> l -> rr -> Pn chain drains
(PE-idle gaps >3.4us re-throttle the HAM clock gate to half rate).
"""

import sys

sys.path.insert(0, "/opt/trn_rl_repo")

from contextlib import ExitStack

import ml_dtypes
import numpy as np

import concourse.bass as bass
import concourse.tile as tile
from concourse.tile import add_dep_helper
from concourse import bacc, bass_utils, mybir

T, B, D, H, E = 1024, 8, 256, 8, 512
N_CORES = 8
AX = 8.0        # fp8 scale on x8 (folded into qt host-side)
AZ = 128.0      # fp8 scale on z8 (logit psum = AX*AZ*lg)
AP = 8192.0     # scale on Pn / out_acc

F32 = mybir.dt.float32
BF16 = mybir.dt.bfloat16
F8 = mybir.dt.float8e4
AF = mybir.ActivationFunctionType
ALU = mybir.AluOpType
DR = mybir.MatmulPerfMode.DoubleRow


def _bcast(ap_row, parts):
    """Partition-broadcast a [1, n] DRAM AP to [parts, n] (step-0 partition)."""
    return bass.AP(
        tensor=ap_row.tensor,
        offset=ap_row.offset,
        ap=[[0, parts], list(ap_row.ap[-1])],
    )


def build_nc(t=T, d=D, h=H, e=E):
    """Build the per-core SPMD program. Returns a compiled Bacc."""
    TC = t // 512   # t chunks (512-wide psum free dim)
    SB = t // 128   # s blocks
    DC = d // 128   # d chunks (contraction for projections)
    DB = d // 128   # d blocks (z free dim / transposed-output partitions)

    sc = 1.0 / (AX * AZ)

    nc = bacc.Bacc("TRN2", target_bir_lowering=False, debug=False)

    qt_d = nc.dram_tensor("qt", [128, DC, t], BF16, kind="ExternalInput").ap()
    gz_d = nc.dram_tensor("gz", [h, 128, DC, d], BF16, kind="ExternalInput").ap()
    mt_d = nc.dram_tensor("mt", [h, 128, DC, d], BF16, kind="ExternalInput").ap()
    w1_d = nc.dram_tensor("w1s", [128, h, DB], F32, kind="ExternalInput").ap()
    cs_d = nc.dram_tensor("cs", [h, d], F32, kind="ExternalInput").ap()
    boc_d = nc.dram_tensor("boc", [128, DB], F32, kind="ExternalInput").ap()
    out_d = nc.dram_tensor("out", [d, t], F32, kind="ExternalOutput").ap()

    with tile.TileContext(nc) as tc, ExitStack() as ctx:
        consts = ctx.enter_context(tc.tile_pool(name="consts", bufs=1))
        wpool = ctx.enter_context(tc.tile_pool(name="wpool", bufs=2))
        hpool = ctx.enter_context(tc.tile_pool(name="hpool", bufs=2))
        qkpool = ctx.enter_context(tc.tile_pool(name="qkpool", bufs=2))
        spool = ctx.enter_context(tc.tile_pool(name="spool", bufs=2))
        at_pool = ctx.enter_context(tc.tile_pool(name="at_pool", bufs=3, space="PSUM"))
        mm_pool = ctx.enter_context(tc.tile_pool(name="mm_pool", bufs=5, space="PSUM"))

        # ---- persistent loads -------------------------------------------
        qt_sb = consts.tile([128, DC, t], BF16)
        nc.sync.dma_start(out=qt_sb[:, 0, :], in_=qt_d[:, 0, :])
        w1_sb = consts.tile([128, h, DB], F32)
        nc.sync.dma_start(out=w1_sb, in_=w1_d)
        boc_sb = consts.tile([128, DB], F32)
        nc.sync.dma_start(out=boc_sb, in_=boc_d)
        out_acc = consts.tile([128, DB, t], F32)
        out_r = out_d.rearrange("(db p) t -> p db t", p=128)

        # ---- PE warm-up: dummy matmuls during the initial DMA wait ------
        scratch = consts.tile([128, 640], BF16)
        nc.vector.memset(scratch, 0.0)
        ps_w = mm_pool.tile([128, 512], F32, tag="mm")
        for _ in range(24):
            nc.tensor.matmul(
                ps_w, scratch[:, :128], scratch[:, 128:640], start=True, stop=True
            )

        gz_sb = [None] * h
        mt_sb = [None] * h
        c_bc = [None] * h
        zT8 = [None] * h
        P32 = [None] * h
        Pnb = [None] * h
        Ex = [None] * h

        def load_head(hh, gate_mm=None):
            gz_sb[hh] = wpool.tile([128, DC, d], BF16, tag="gz", name="gz_sb")
            nc.sync.dma_start(out=gz_sb[hh], in_=gz_d[hh])
            mt_sb[hh] = wpool.tile([128, DC, d], BF16, tag="mt", name="mt_sb")
            mm = nc.sync.dma_start(out=mt_sb[hh], in_=mt_d[hh])
            c_bc[hh] = wpool.tile([128, d], F32, tag="c", name="c_bc")
            cc = nc.gpsimd.dma_start(
                out=c_bc[hh], in_=_bcast(cs_d[hh][None, :], 128)
            )
            if gate_mm is not None:
                for g in (mm, cc):
                    add_dep_helper(
                        g.ins, gate_mm.ins, reason="defer bulk load past cold start"
                    )

        def z_group(hh, db, tch):
            """One psum-group of the z projection: z = x@(s*G) + w1 -> fp8."""
            tsl = slice(tch * 512, (tch + 1) * 512)
            ps_z = mm_pool.tile([128, 512], F32, tag="mm")
            first = None
            for dc in range(DC):
                mm = nc.tensor.matmul(
                    ps_z,
                    gz_sb[hh][:, dc, db * 128 : (db + 1) * 128],
                    qt_sb[:, dc, tsl],
                    start=(dc == 0),
                    stop=(dc == DC - 1),
                )
                first = first or mm
            nc.vector.tensor_scalar_add(
                zT8[hh][:, db, tsl], ps_z, w1_sb[:, hh, db : db + 1]
            )
            return first

        def p_group(hh, sb):
            """One psum-group of the P projection: P32 = x @ M^T + c."""
            ssl = slice(sb * 128, (sb + 1) * 128)
            pp = mm_pool.tile([128, 512], F32, tag="mm")
            for dc in range(DC):
                nc.tensor.matmul(
                    pp[:, :d],
                    qt_sb[:, dc, ssl],
                    mt_sb[hh][:, dc, :],
                    start=(dc == 0),
                    stop=(dc == DC - 1),
                )
            nc.vector.tensor_add(P32[hh][:, sb, :], pp[:, :d], c_bc[hh])

        def av_group(hh, dt, tch):
            """One psum-group of the AV matmul (bf16): out^T += Pn^T x Ex."""
            dsl = slice(dt * 128, (dt + 1) * 128)
            tsl = slice(tch * 512, (tch + 1) * 512)
            ot = mm_pool.tile([128, 512], F32, tag="mm")
            for sb in range(SB):
                nc.tensor.matmul(
                    ot,
                    Pnb[hh][:, sb, dsl],
                    Ex[hh][:, sb, tsl],
                    start=(sb == 0),
                    stop=(sb == SB - 1),
                )
            if hh == 0:
                nc.scalar.activation(out_acc[:, dt, tsl], ot, AF.Copy)
            else:
                nc.vector.tensor_add(out_acc[:, dt, tsl], out_acc[:, dt, tsl], ot)

        # ---- prologue: head 0 z/P projections + the shared x8 cast ------
        load_head(0)
        nc.sync.dma_start(out=qt_sb[:, 1, :], in_=qt_d[:, 1, :])
        zT8[0] = qkpool.tile([128, DB, t], F8, tag="zT", name="zT8")
        first_mm0 = None
        for db in range(DB):
            for tch in range(TC):
                mm = z_group(0, db, tch)
                first_mm0 = first_mm0 or mm
        x8 = consts.tile([128, DC, t], F8)
        for dc in range(DC):
            nc.vector.tensor_scalar_mul(x8[:, dc, :], qt_sb[:, dc, :], 1.0)
        P32[0] = hpool.tile([128, SB, d], F32, tag="P32", name="P32")
        for sb in range(2):
            p_group(0, sb)

        for hh in range(h):
            if hh + 1 < h:
                load_head(hh + 1, gate_mm=first_mm0 if hh == 0 else None)
                zT8[hh + 1] = qkpool.tile([128, DB, t], F8, tag="zT", name="zT8")
                P32[hh + 1] = hpool.tile([128, SB, d], F32, tag="P32", name="P32")

            # filler matmul groups to interleave with this head's scores:
            # previous head's AV + next head's z/P projections
            big = []     # AV groups: 8 matmuls each
            small = []   # z/P projection groups: 2 matmuls each
            if hh > 0:
                for dt in range(DB):
                    for tch in range(TC):
                        big.append(lambda dt=dt, tch=tch: av_group(hh - 1, dt, tch))
            if hh == 0:
                for sb in range(2, SB):
                    small.append(lambda sb=sb: p_group(0, sb))
            if hh + 1 < h:
                for db in range(DB):
                    for tch in range(TC):
                        small.append(lambda db=db, tch=tch: z_group(hh + 1, db, tch))
                for sb in range(SB):
                    small.append(lambda sb=sb: p_group(hh + 1, sb))
            # weave big groups evenly among the small ones so every score
            # round gets enough PE filler to cover the ACT exp pace
            fillers = []
            nb, ns = len(big), len(small)
            bi = si = 0
            for k in range(nb + ns):
                if bi * (nb + ns) <= k * nb and bi < nb:
                    fillers.append(big[bi]); bi += 1
                elif si < ns:
                    fillers.append(small[si]); si += 1
                else:
                    fillers.append(big[bi]); bi += 1

            # ---- scores (fp8 DR) -> exp on ACT (+accum l), interleaved --
            Ex[hh] = hpool.tile([128, SB, t], BF16, tag="Ex", name="Ex")
            lsum2 = spool.tile([128, SB, TC], F32)
            nf = len(fillers)
            fi = 0
            for sb in range(SB):
                ssl = slice(sb * 128, (sb + 1) * 128)
                for tch in range(TC):
                    tsl = slice(tch * 512, (tch + 1) * 512)
                    at = at_pool.tile([128, 512], F32, tag="at")
                    nc.tensor.matmul(
                        at,
                        zT8[hh][:, :, ssl],
                        x8[:, :, tsl],
                        start=True,
                        stop=True,
                        perf_mode=DR,
                    )
                    nc.scalar.activation(
                        Ex[hh][:, sb, tsl],
                        at,
                        AF.Exp,
                        scale=sc,
                        accum_out=lsum2[:, sb, tch : tch + 1],
                    )
                quota = (nf * (sb + 1)) // SB
                while fi < quota:
                    fillers[fi]()
                    fi += 1
            while fi < nf:
                fillers[fi]()
                fi += 1

            # ---- softmax denominators: rr2 = AP / l ---------------------
            ls = spool.tile([128, SB], F32)
            lsS = spool.tile([128, SB], F32)
            rr2 = spool.tile([128, SB], F32)
            nc.vector.tensor_add(ls, lsum2[:, :, 0], lsum2[:, :, 1])
            nc.vector.tensor_scalar_mul(lsS, ls, 1.0 / AP)
            nc.vector.reciprocal(rr2, lsS)

            # ---- Pnb (bf16 stationary operand of AV) on DVE -------------
            Pnb[hh] = hpool.tile([128, SB, d], BF16, tag="Pnb", name="Pnb")
            for sb in range(SB):
                nc.vector.tensor_scalar_mul(
                    Pnb[hh][:, sb, :], P32[hh][:, sb, :], rr2[:, sb : sb + 1]
                )

        # ---- epilogue: last head's AV; fold + store per d-block ---------
        for dt in range(DB):
            for tch in range(TC):
                av_group(h - 1, dt, tch)
            nc.vector.tensor_scalar(
                out_acc[:, dt, :],
                out_acc[:, dt, :],
                boc_sb[:, dt : dt + 1],
                1.0 / AP,
                op0=ALU.add,
                op1=ALU.mult,
            )
            nc.sync.dma_start(out=out_r[:, dt, :], in_=out_acc[:, dt, :])

    nc.compile()
    return nc


_NC_CACHE = {}


def _get_nc(shape_key):
    if shape_key not in _NC_CACHE:
        _NC_CACHE[shape_key] = build_nc(*shape_key)
    return _NC_CACHE[shape_key]


def _pmajor(a, last):
    """[..., C*128, last] -> [..., 128, C, last] partition-major layout."""
    lead = a.shape[:-2]
    c = a.shape[-2] // 128
    return np.ascontiguousarray(
        a.reshape(*lead, c, 128, last).swapaxes(-3, -2)
    )


def _prep_inputs(Q, Wq, bq, Wk, bk, Wv, bv, Wo, bo):
    t, b, d = Q.shape
    h, e, _ = Wq.shape
    s = np.float32(1.0 / np.sqrt(e))
    bf = ml_dtypes.bfloat16
    Q = np.asarray(Q, np.float32)
    Wq = np.asarray(Wq, np.float32)
    Wk = np.asarray(Wk, np.float32)
    Wv = np.asarray(Wv, np.float32)
    Wo = np.asarray(Wo, np.float32)
    bk = np.asarray(bk, np.float32)
    bv = np.asarray(bv, np.float32)
    bo = np.asarray(bo, np.float32)
    # [B, 128, DC, T] partition-major AX * x^T per batch
    qt_all = _pmajor((Q * AX).transpose(1, 2, 0).astype(bf), t)
    # z side: Gz = (AZ*s/AX) * (Wk^T @ Wq),  w1 = AZ*s*Wq^T@bk
    gzs = np.stack([(AZ * s / AX) * (Wk[hh].T @ Wq[hh]) for hh in range(h)])
    gz = _pmajor(gzs.astype(bf), d)
    w1 = np.stack([(AZ * s) * (Wq[hh].T @ bk[hh]) for hh in range(h)])
    # P side: M_h = Wo_h @ Wv_h; mt stores M_h^T/AX partition-major over d'
    Wo_heads = Wo.reshape(d, h, e)
    mts = np.stack([(Wo_heads[:, hh, :] @ Wv[hh]).T / AX for hh in range(h)])
    mt = _pmajor(mts.astype(bf), d)
    cs = np.stack([bv[hh] @ Wo_heads[:, hh, :].T for hh in range(h)])
    shared = {
        "gz": gz,
        "mt": mt,
        "w1s": np.ascontiguousarray(w1.reshape(h, -1, 128).transpose(2, 0, 1)),
        "cs": np.ascontiguousarray(cs.astype(np.float32)),
        "boc": np.ascontiguousarray((bo * AP).reshape(-1, 128).T.astype(np.float32)),
    }
    in_maps = [
        {"qt": np.ascontiguousarray(qt_all[bb]), **shared} for bb in range(b)
    ]
    return in_maps, (t, d, h, e)


def kernel(Q, Wq, bq, Wk, bk, Wv, bv, Wo, bo, _trace=False):
    in_maps, (t, d, h, e) = _prep_inputs(Q, Wq, bq, Wk, bk, Wv, bv, Wo, bo)
    nc = _get_nc((t, d, h, e))
    res = bass_utils.run_bass_kernel_spmd(
        nc, in_maps, core_ids=list(range(len(in_maps))), trace=_trace
    )
    # per-core output is out^T [D, T]; transpose back and stack over batch
    out = np.stack(
        [res.results[bb]["out"].T for bb in range(len(in_maps))], axis=1
    )
    if _trace:
        kernel.last_results = res
    return np.ascontiguousarray(out.astype(np.float32))
